# revision 57
# baseline (speedup 1.0000x reference)
"""Causal single-head attention (B=4, S=4096, D=1024, d_key=64) on 8 trn2 cores.

Sharding: 8 cores = 4 batches x 2 KEY-halves. Core (b, h) holds ALL 4096 query
rows of batch b but only the key/value 128-row blocks {j : j % 2 == h} (2048
keys, interleaved for causal balance). Each core computes the partial softmax
accumulator (unnormalized numerator + denominator row) of every query row over
its own key half; the HOST adds the two halves of each pair and normalizes.
No cross-core communication, and K/V raw loads + projections are not
replicated (the baseline replicated both).

DMA diet: queries, keys AND values stream in as fp8 e3m4 (4 mantissa bits);
the projection weights Wq/Wk are pre-scaled by 64 on the host so their range
suits e3m4, and the 1/64^2 is folded into the softmax exp scale. Wv stays
bf16 (mixed e3m4-lhsT x bf16-rhs matmuls work). Measured end-to-end l2
rel-err 1.55e-2 vs fp64 (gate 2e-2): ~1.0e-2 from the q/k score path,
~1.1e-2 from the V path, in quadrature.

Device kernel (identical SPMD program; per-core differences are input data):
  1. Project qT [64, 4096] and kT [64, 2048] (weights stationary, e3m4 data,
     fp32 PSUM, stored bf16) and v-natural [128, 65]-blocks (data stationary
     -> natural PV layout; col 64 is a ones column for the denominator).
     Projections are split into per-matmul thunks and woven one-or-two at a
     time between attention units (the `bg` queue) so the in-order PE stays
     fed while stage DMAs pace the input.
  2. CHUNK-major attention, two chunks interleaved per pair (each with its
     own PSUM accumulator bank): for q chunk c (256 rows), own-key blocks
     m=0..c (the packed block m maps to global block 2m+h, so the count and
     boundary structure are core-independent): score matmuls in units of up
     to 4 blocks -> one ACT exp per unit -> boundary mask (a single constant
     [128,256] tile, only block m==c needs it) -> PV matmuls accumulate the
     whole chunk in one PSUM tile [65, 256]. PV emission is DEFERRED behind
     a 6-unit window so the in-order PE never parks on a PV waiting for its
     exp -- later units' scores fill the ACT latency instead.
  3. Per chunk the accumulator is copied to SBUF and DMAd out via the POOL
     DGE (last two chunks via HWDGE); host sums the two half outputs of each
     batch pair, normalizes by the denominator row, and transposes.
"""

import numpy as np

import concourse.mybir as mybir
import concourse.tile as tile
from concourse import bacc
from concourse.bass_utils import run_bass_kernel_spmd

B, S, D, DK = 4, 4096, 1024, 64
NCORES = 8
CH = 256  # query rows per chunk
NCH = 16  # chunks per core (all rows)
KB = 2048  # own keys per core
JB = 128  # key block
NKB = KB // JB  # 16 own key blocks
DC = D // 128  # 8 contraction chunks
F32 = mybir.dt.float32
BF16 = mybir.dt.bfloat16
E3 = mybir.dt.float8e3
WSCALE = 64.0  # host pre-scales Wq/Wk by this; folded into exp scale
SCALE = 0.125 / (WSCALE * WSCALE)

_prog_cache = {}


def _build(variant):
    causal = variant == "causal"
    nkq = [c + 1 if causal else NKB for c in range(NCH)]  # own blocks/chunk

    nc = bacc.Bacc("TRN2", target_bir_lowering=False, debug=False,
                   num_devices=NCORES)

    qt_d = nc.declare_dram_parameter("qt", [D, S], E3, isOutput=False)
    kt_d = nc.declare_dram_parameter("kt", [D, KB], E3, isOutput=False)
    vt_d = nc.declare_dram_parameter("vt", [D, KB], E3, isOutput=False)
    wq_d = nc.declare_dram_parameter("wq", [D, DK], E3, isOutput=False)
    wk_d = nc.declare_dram_parameter("wk", [D, DK], E3, isOutput=False)
    wv_d = nc.declare_dram_parameter("wv", [D, DK], BF16, isOutput=False)
    eye_d = nc.declare_dram_parameter("eye", [128, 128], BF16,
                                      isOutput=False)
    if causal:
        mask_d = nc.declare_dram_parameter("maskb", [JB, CH], BF16,
                                           isOutput=False)
    # raw transposed partial accumulators (+denominator row); host combines
    out_d = nc.declare_dram_parameter("out", [NCH, DK + 1, CH], F32,
                                      isOutput=True)

    NSQ = S // 512  # 8 column groups of 512 for q
    NSK = KB // 512  # 4 groups for k/v

    qt3 = qt_d.rearrange("(o p) s -> p o s", p=128)
    kt3 = kt_d.rearrange("(o p) s -> p o s", p=128)
    vt3 = vt_d.rearrange("(o p) s -> p o s", p=128)

    with tile.TileContext(nc) as tc:
        with (
            tc.tile_pool(name="const", bufs=1) as const,
            tc.tile_pool(name="res", bufs=1) as res,
            tc.tile_pool(name="stage", bufs=12) as stage,
            tc.tile_pool(name="pwork", bufs=10) as pwork,
            tc.tile_pool(name="ps_mm", bufs=2, space="PSUM") as ps_mm,
            tc.tile_pool(name="ps_s", bufs=2, space="PSUM") as ps_s,
            tc.tile_pool(name="ps_o", bufs=2, space="PSUM") as ps_o,
        ):
            def stage_load(src3, sc, dt, splits=2):
                """Split-group DMAs so the first matmuls start early."""
                w = DC // splits
                sts = []
                for hh in range(splits):
                    st = stage.tile([128, w, 512], dt, tag="stage",
                                    name=f"st{hh}")
                    nc.sync.dma_start(
                        st[:],
                        src3[:, w * hh:w * (hh + 1), sc * 512:(sc + 1) * 512])
                    sts.append(st)
                return sts

            bg = []  # background projection thunks, woven between attn units

            def project_q(sc, sts=None, defer=False):
                """Q group projected DATA-stationary (4 cyc/row) to natural
                [row, 64] blocks, then PE-transposed ([128,64] -> [64,128]
                via the identity rhs) into the usual qT tile -- 2560 PE
                cycles per 512 rows instead of weights-stationary 4096."""
                if sts is None:
                    sts = stage_load(qt3, sc, E3)
                w = DC // len(sts)
                box = {}

                def mm(sb):
                    if sb == 0:
                        box["ps"] = ps_mm.tile([128, 4, DK], F32, tag="mm",
                                               name="ps_qn")
                    for dc in range(DC):
                        nc.tensor.matmul(
                            box["ps"][:, sb, :],
                            sts[dc // w][:, dc % w,
                                         sb * 128:(sb + 1) * 128],
                            wq_sb[:, dc, :],
                            start=(dc == 0), stop=(dc == DC - 1))

                def cpn():
                    nc.vector.tensor_copy(qng[sc][:], box["ps"][:])

                def xp():
                    box["pt"] = ps_mm.tile([DK, 4, 128], BF16, tag="mm",
                                           name="ps_qt")
                    for b in range(4):
                        nc.tensor.transpose(box["pt"][:, b, :],
                                            qng[sc][:, b, :], eye_sb[:])

                def cpt():
                    nc.vector.tensor_copy(qts[sc][:], box["pt"][:])

                steps = [lambda sb=sb: mm(sb) for sb in range(4)]
                steps += [cpn, xp, cpt]
                for st in steps:
                    if defer:
                        bg.append((("q", sc), st))
                    else:
                        st()

            def project_qk(kind, src3, w_sb, dst, sc, sts=None, defer=False):
                """One 512-column group: 8 accumulating matmuls (weights
                stationary); psum copied to the bf16 qT/kT tile."""
                if sts is None:
                    sts = stage_load(src3, sc, E3)
                w = DC // len(sts)
                box = {}

                def mm(dc):
                    if dc == 0:
                        box["ps"] = ps_mm.tile([DK, 512], F32, tag="mm",
                                               name="ps_qk")
                    nc.tensor.matmul(box["ps"][:], w_sb[:, dc, :],
                                     sts[dc // w][:, dc % w, :],
                                     start=(dc == 0), stop=(dc == DC - 1))

                def cp():
                    nc.vector.tensor_copy(dst[:], box["ps"][:])

                steps = [lambda dc=dc: mm(dc) for dc in range(DC)] + [cp]
                for st in steps:
                    if defer:
                        bg.append(((kind, sc), st))
                    else:
                        st()

            def project_v(sc, sts=None, defer=False):
                """V projected directly to natural [s, c] blocks: lhsT is the
                staged data chunk, rhs the weights -> out [128 s, 64 c], which
                is exactly the PV lhsT layout."""
                if sts is None:
                    sts = stage_load(vt3, sc, E3)
                w = DC // len(sts)
                box = {}

                def mm(sb):
                    if sb == 0:
                        box["ps"] = ps_mm.tile([128, 4, DK], F32, tag="mm",
                                               name="ps_v")
                    for dc in range(DC):
                        nc.tensor.matmul(
                            box["ps"][:, sb, :],
                            sts[dc // w][:, dc % w,
                                         sb * 128:(sb + 1) * 128],
                            wv_sb[:, dc, :],
                            start=(dc == 0), stop=(dc == DC - 1))

                def cp():
                    nc.vector.tensor_copy(vgs[sc][:, :, 0:DK], box["ps"][:])
                    nc.vector.memset(vgs[sc][:, :, DK:DK + 1], 1.0)

                steps = [lambda sb=sb: mm(sb) for sb in range(4)] + [cp]
                for st in steps:
                    if defer:
                        bg.append((("v", sc), st))
                    else:
                        st()

            # PE warm-up in the initial DMA shadow
            warm = const.tile([128, 512], BF16, tag="warm")
            nc.vector.memset(warm[:], 0.0)
            for _ in range(4):
                wps = ps_mm.tile([DK, 512], F32, tag="mm", name="wps")
                nc.tensor.matmul(wps[:], warm[:, 0:DK], warm[:],
                                 start=True, stop=True)
            wq_sb = const.tile([128, DC, DK], E3, tag="wq")
            wk_sb = const.tile([128, DC, DK], E3, tag="wk")
            wv_sb = const.tile([128, DC, DK], BF16, tag="wv")
            head_q0 = stage_load(qt3, 0, E3)
            nc.sync.dma_start(wq_sb[:], wq_d.rearrange("(o p) c -> p o c", p=128))
            nc.sync.dma_start(wk_sb[:], wk_d.rearrange("(o p) c -> p o c", p=128))
            nc.sync.dma_start(wv_sb[:], wv_d.rearrange("(o p) c -> p o c", p=128))
            head_k0 = stage_load(kt3, 0, E3)
            head_v0 = stage_load(vt3, 0, E3)
            eye_sb = const.tile([128, 128], BF16, tag="eye")
            nc.sync.dma_start(eye_sb[:], eye_d[:])
            if causal:
                msk_sb = const.tile([JB, CH], BF16, tag="msk")
                nc.sync.dma_start(msk_sb[:], mask_d[:])

            # qT tiles [64, 512] bf16 (2 chunks per tile)
            qts = [res.tile([DK, 512], BF16, tag=f"qt{sc}", name=f"qt{sc}")
                   for sc in range(NSQ)]
            # natural-layout staging for the q transposes, one per group
            qng = [res.tile([128, 4, DK], BF16, tag=f"qn{sc}", name=f"qn{sc}")
                   for sc in range(NSQ)]
            # kT tiles [64, 512] (4 own key blocks per tile)
            kts = [res.tile([DK, 512], BF16, tag=f"kt{sc}", name=f"kt{sc}")
                   for sc in range(NSK)]
            # v natural (+ones col): per 512-group, 4 blocks of [128, 65]
            vgs = [res.tile([128, 4, DK + 1], BF16, tag=f"vg{sc}",
                            name=f"vg{sc}")
                   for sc in range(NSK)]

            def q_rhs(c):
                return qts[c // 2][:, (c % 2) * CH:(c % 2 + 1) * CH]

            def emit_scores(c, m0, nb):
                """Scores + exp (+boundary mask) for one unit; PV deferred so
                the in-order PE isn't parked on a PV that waits for this
                unit's exp -- later scores slot into the exp latency."""
                nb_tot = nkq[c]
                s_ps = ps_s.tile([128, nb, CH], F32, tag="s", name=f"s{nb}")
                for i in range(nb):
                    m = m0 + i
                    nc.tensor.matmul(
                        s_ps[:, i, :],
                        kts[m // 4][:, (m % 4) * JB:(m % 4 + 1) * JB],
                        q_rhs(c), start=True, stop=True)
                if bg:
                    bg.pop(0)[1]()
                    if len(bg) > 10:
                        bg.pop(0)[1]()
                p_sb = pwork.tile([128, nb, CH], BF16, tag="p",
                                  name=f"p{nb}")
                nc.scalar.activation(p_sb[:], s_ps[:],
                                     mybir.ActivationFunctionType.Exp,
                                     scale=SCALE)
                if causal and m0 + nb == nb_tot:
                    # boundary block is always the chunk's last block
                    nc.vector.tensor_mul(p_sb[:, nb - 1, :],
                                         p_sb[:, nb - 1, :], msk_sb[:])
                return (c, m0, nb, p_sb)

            def emit_pv(item, ops, started, left, done):
                c, m0, nb, p_sb = item
                left[c] -= 1
                for i in range(nb):
                    m = m0 + i
                    nc.tensor.matmul(
                        ops[c][:], vgs[m // 4][:, m % 4, :], p_sb[:, i, :],
                        start=(not started[c] and i == 0),
                        stop=(left[c] == 0 and i == nb - 1))
                started[c] = True
                if left[c] == 0:
                    done.append(c)

            def epilogue(c, o_ps):
                # POOL DGE so result stores don't head-of-line block the SP
                # sequencer issuing input stage loads
                o_sb = pwork.tile([DK + 1, CH], F32, tag="osb", name="o_sb")
                nc.vector.tensor_copy(o_sb[:], o_ps[:])
                eng = nc.sync if c >= NCH - 2 else nc.gpsimd
                eng.dma_start(out_d[c], o_sb[:])

            def chunk_pair(c0, c1):
                """Interleave the score/exp/PV units of two chunks so one
                chunk's PE work hides the other's ACT-exp latency (each chunk
                accumulates in its own PSUM bank)."""
                cs = [c for c in (c0, c1) if c is not None]
                units = {c: [(m0, min(4, nkq[c] - m0))
                             for m0 in range(0, nkq[c], 4)] for c in cs}
                # boundary (masked) unit first, so each chunk's closing
                # chain has no mask op and the masked exp starts earliest
                for c in cs:
                    units[c] = units[c][-1:] + units[c][:-1]
                ops = {c: ps_o.tile([DK + 1, CH], F32, tag="o",
                                    name=f"o{c % 2}") for c in cs}
                started = {c: False for c in cs}
                left = {c: len(units[c]) for c in cs}
                pend = []
                done = []
                nu = max(len(units[c]) for c in cs)
                for u in range(nu):
                    for c in cs:
                        if u < len(units[c]):
                            m0, nb = units[c][u]
                            pend.append(emit_scores(c, m0, nb))
                            while len(pend) > 6:
                                emit_pv(pend.pop(0), ops, started, left, done)
                while pend:
                    emit_pv(pend.pop(0), ops, started, left, done)
                for c in done:
                    epilogue(c, ops[c])

            # projection prefetch schedule: kT pair tile m+1 (k group
            # (m+1)//4) is needed by chunk m, so k group g feeds chunks
            # >= 4g-1; v group g feeds >= 4g; q group g feeds >= 2g
            pre = {c: [] for c in range(NCH)}
            for g in range(1, NSK):
                pre[max(0, 4 * g - 3)] += [("k", g), ("v", g)]
            for g in range(1, NSQ):  # q groups 1..7 needed at chunk 2g
                pre[max(0, 2 * g - 3)] += [("q", g)]

            project_q(0, sts=head_q0)
            project_qk("k", kt3, wk_sb, kts[0], 0, sts=head_k0)
            project_v(0, sts=head_v0)
            for c0 in range(0, NCH, 2):
                c1 = c0 + 1
                # groups the current pair depends on must be fully emitted
                needed = {("q", g) for g in range(c1 // 2 + 1)}
                needed |= {("k", g) for g in range(c1 // 4 + 1)}
                needed |= {("v", g) for g in range(c1 // 4 + 1)}
                while any(k in needed for k, _ in bg):
                    bg.pop(0)[1]()
                # stage + enqueue projections for upcoming chunks; their
                # matmuls are woven between this pair's attention units
                for c in (c0, c1):
                    for kind, g in pre[c]:
                        if kind == "q":
                            project_q(g, defer=True)
                        elif kind == "k":
                            project_qk("k", kt3, wk_sb, kts[g], g, defer=True)
                        else:
                            project_v(g, defer=True)
                chunk_pair(c0, c1)
            while bg:
                bg.pop(0)[1]()

    nc.compile()
    return nc


def _get_prog(variant):
    if variant not in _prog_cache:
        _prog_cache[variant] = _build(variant)
    return _prog_cache[variant]


def kernel(queries, keys, values, Wq, Wk, Wv, mask):
    import ml_dtypes  # noqa: F401  registers numpy bfloat16/fp8

    bf16 = np.dtype(mybir.dt.np(BF16))
    e3m4 = np.dtype(mybir.dt.np(E3))
    queries = np.asarray(queries, dtype=np.float32)
    keys = np.asarray(keys, dtype=np.float32)
    values = np.asarray(values, dtype=np.float32)
    mask_np = np.asarray(mask)

    causal = bool(np.array_equal(
        mask_np != 0, np.tril(np.ones((S, S), dtype=bool))))
    full = bool((mask_np != 0).all()) if not causal else False
    if not (causal or full):
        raise NotImplementedError("general mask not supported")
    variant = "causal" if causal else "full"

    qt = np.ascontiguousarray(queries.transpose(0, 2, 1)).astype(e3m4)
    kt = np.ascontiguousarray(keys.transpose(0, 2, 1)).astype(e3m4)
    vt = np.ascontiguousarray(values.transpose(0, 2, 1)).astype(e3m4)
    wq = np.ascontiguousarray(
        np.asarray(Wq, dtype=np.float32).T * WSCALE).astype(e3m4)
    wk = np.ascontiguousarray(
        np.asarray(Wk, dtype=np.float32).T * WSCALE).astype(e3m4)
    wv = np.ascontiguousarray(np.asarray(Wv, dtype=np.float32).T).astype(bf16)

    in_maps = []
    for core in range(NCORES):
        b, h = divmod(core, 2)
        ksel = np.ascontiguousarray(
            kt[b].reshape(D, S // JB, JB)[:, h::2, :].reshape(D, KB))
        vsel = np.ascontiguousarray(
            vt[b].reshape(D, S // JB, JB)[:, h::2, :].reshape(D, KB))
        m = {"qt": qt[b], "kt": ksel, "vt": vsel,
             "wq": wq, "wk": wk, "wv": wv,
             "eye": np.eye(128, dtype=np.float32).astype(bf16)}
        if variant == "causal":
            i = np.arange(CH)[None, :]
            j = np.arange(JB)[:, None]
            m["maskb"] = ((i - j - JB * h) >= 0).astype(np.float32).astype(bf16)
        in_maps.append(m)

    nc = _get_prog(variant)
    res = run_bass_kernel_spmd(nc, in_maps, list(range(NCORES)))

    out = np.empty((B, S, DK), dtype=np.float32)
    ov = out.reshape(B, NCH, CH, DK)
    for b in range(B):
        r0 = res.results[2 * b]["out"]  # [NCH, DK+1, CH]
        r1 = res.results[2 * b + 1]["out"]
        tot = r0.astype(np.float64) + r1.astype(np.float64)
        ov[b] = (tot[:, :DK, :] / tot[:, DK:DK + 1, :]).transpose(0, 2, 1)
    return out


if __name__ == "__main__":
    rng = np.random.default_rng(0)
    q = rng.standard_normal((B, S, D), dtype=np.float32)
    k = rng.standard_normal((B, S, D), dtype=np.float32)
    v = rng.standard_normal((B, S, D), dtype=np.float32)
    sc = 1.0 / np.sqrt(D)
    wq = rng.uniform(-sc, sc, (DK, D)).astype(np.float32)
    wk = rng.uniform(-sc, sc, (DK, D)).astype(np.float32)
    wv = rng.uniform(-sc, sc, (DK, D)).astype(np.float32)
    msk = np.tril(np.ones((S, S), dtype=np.int32))
    out = kernel(queries=q, keys=k, values=v, Wq=wq, Wk=wk, Wv=wv, mask=msk)
    print("out", out.shape, out.dtype, float(np.abs(out).mean()))


# revision 59
# speedup vs baseline: 1.0582x; 1.0582x over previous
"""Causal single-head attention (B=4, S=4096, D=1024, d_key=64) on 8 trn2 cores.

Sharding: 8 cores = 4 batches x 2 KEY-halves. Core (b, h) holds ALL 4096 query
rows of batch b but only the key/value 128-row blocks {j : j % 2 == h} (2048
keys, interleaved for causal balance). Each core computes the partial softmax
accumulator (unnormalized numerator + denominator row) of every query row over
its own key half; the HOST adds the two halves of each pair and normalizes.
No cross-core communication, and K/V raw loads + projections are not
replicated (the baseline replicated both).

DMA diet: queries, keys AND values stream in as fp8 e3m4 (4 mantissa bits);
the projection weights Wq/Wk are pre-scaled by 64 on the host so their range
suits e3m4, and the 1/64^2 is folded into the softmax exp scale. Wv stays
bf16 (mixed e3m4-lhsT x bf16-rhs matmuls work). Measured end-to-end l2
rel-err 1.55e-2 vs fp64 (gate 2e-2): ~1.0e-2 from the q/k score path,
~1.1e-2 from the V path, in quadrature.

Device kernel (identical SPMD program; per-core differences are input data):
  1. Project qT [64, 4096] and kT [64, 2048] (weights stationary, e3m4 data,
     fp32 PSUM, stored bf16) and v-natural [128, 65]-blocks (data stationary
     -> natural PV layout; col 64 is a ones column for the denominator).
     Projections are split into per-matmul thunks and woven one-or-two at a
     time between attention units (the `bg` queue) so the in-order PE stays
     fed while stage DMAs pace the input.
  2. CHUNK-major attention, two chunks interleaved per pair (each with its
     own PSUM accumulator bank): for q chunk c (256 rows), own-key blocks
     m=0..c (the packed block m maps to global block 2m+h, so the count and
     boundary structure are core-independent): score matmuls in units of up
     to 4 blocks -> one ACT exp per unit -> boundary mask (a single constant
     [128,256] tile, only block m==c needs it) -> PV matmuls accumulate the
     whole chunk in one PSUM tile [65, 256]. PV emission is DEFERRED behind
     a 6-unit window so the in-order PE never parks on a PV waiting for its
     exp -- later units' scores fill the ACT latency instead.
  3. Per chunk the accumulator is copied to SBUF and DMAd out via the POOL
     DGE (last two chunks via HWDGE); host sums the two half outputs of each
     batch pair, normalizes by the denominator row, and transposes.
"""

import numpy as np

import concourse.mybir as mybir
import concourse.tile as tile
from concourse import bacc
from concourse.bass_utils import run_bass_kernel_spmd

B, S, D, DK = 4, 4096, 1024, 64
NCORES = 8
CH = 256  # query rows per chunk
NCH = 16  # chunks per core (all rows)
KB = 2048  # own keys per core
JB = 128  # key block
NKB = KB // JB  # 16 own key blocks
DC = D // 128  # 8 contraction chunks
F32 = mybir.dt.float32
BF16 = mybir.dt.bfloat16
E3 = mybir.dt.float8e3
WSCALE = 64.0  # host pre-scales Wq/Wk by this; folded into exp scale
SCALE = 0.125 / (WSCALE * WSCALE)

_prog_cache = {}


def _build(variant):
    causal = variant == "causal"
    nkq = [c + 1 if causal else NKB for c in range(NCH)]  # own blocks/chunk

    nc = bacc.Bacc("TRN2", target_bir_lowering=False, debug=False,
                   num_devices=NCORES)

    qt_d = nc.declare_dram_parameter("qt", [D, S], E3, isOutput=False)
    kt_d = nc.declare_dram_parameter("kt", [D, KB], E3, isOutput=False)
    vt_d = nc.declare_dram_parameter("vt", [D, KB], E3, isOutput=False)
    wq_d = nc.declare_dram_parameter("wq", [D, DK], E3, isOutput=False)
    wk_d = nc.declare_dram_parameter("wk", [D, DK], E3, isOutput=False)
    wv_d = nc.declare_dram_parameter("wv", [D, DK], BF16, isOutput=False)
    eye_d = nc.declare_dram_parameter("eye", [128, 128], BF16,
                                      isOutput=False)
    if causal:
        mask_d = nc.declare_dram_parameter("maskb", [JB, CH], BF16,
                                           isOutput=False)
    # raw transposed partial accumulators (+denominator row); host combines
    out_d = nc.declare_dram_parameter("out", [NCH, DK + 1, CH], F32,
                                      isOutput=True)

    NSQ = S // 512  # 8 column groups of 512 for q
    NSK = KB // 512  # 4 groups for k/v

    qt3 = qt_d.rearrange("(o p) s -> p o s", p=128)
    kt3 = kt_d.rearrange("(o p) s -> p o s", p=128)
    vt3 = vt_d.rearrange("(o p) s -> p o s", p=128)

    with tile.TileContext(nc) as tc:
        with (
            tc.tile_pool(name="const", bufs=1) as const,
            tc.tile_pool(name="res", bufs=1) as res,
            tc.tile_pool(name="stage", bufs=12) as stage,
            tc.tile_pool(name="pwork", bufs=10) as pwork,
            tc.tile_pool(name="ps_mm", bufs=2, space="PSUM") as ps_mm,
            tc.tile_pool(name="ps_s", bufs=2, space="PSUM") as ps_s,
            tc.tile_pool(name="ps_o", bufs=2, space="PSUM") as ps_o,
        ):
            def stage_load(src3, sc, dt, splits=2):
                """Split-group DMAs so the first matmuls start early."""
                w = DC // splits
                sts = []
                for hh in range(splits):
                    st = stage.tile([128, w, 512], dt, tag="stage",
                                    name=f"st{hh}")
                    nc.sync.dma_start(
                        st[:],
                        src3[:, w * hh:w * (hh + 1), sc * 512:(sc + 1) * 512])
                    sts.append(st)
                return sts

            bg = []  # background projection thunks, woven between attn units

            def project_q(sc, sts=None, defer=False):
                """Q group projected DATA-stationary (4 cyc/row) to natural
                [row, 64] blocks, then PE-transposed ([128,64] -> [64,128]
                via the identity rhs) into the usual qT tile -- 2560 PE
                cycles per 512 rows instead of weights-stationary 4096."""
                if sts is None:
                    sts = stage_load(qt3, sc, E3)
                w = DC // len(sts)
                box = {}

                def mm(sb):
                    if sb == 0:
                        box["ps"] = ps_mm.tile([128, 4, DK], F32, tag="mm",
                                               name="ps_qn")
                    for dc in range(DC):
                        nc.tensor.matmul(
                            box["ps"][:, sb, :],
                            sts[dc // w][:, dc % w,
                                         sb * 128:(sb + 1) * 128],
                            wq_sb[:, dc, :],
                            start=(dc == 0), stop=(dc == DC - 1))

                def cpn():
                    nc.vector.tensor_copy(qng[sc][:], box["ps"][:])

                def xp():
                    box["pt"] = ps_mm.tile([DK, 4, 128], BF16, tag="mm",
                                           name="ps_qt")
                    for b in range(4):
                        nc.tensor.transpose(box["pt"][:, b, :],
                                            qng[sc][:, b, :], eye_sb[:])

                def cpt():
                    nc.vector.tensor_copy(qts[sc][:], box["pt"][:])

                steps = [lambda sb=sb: mm(sb) for sb in range(4)]
                steps += [cpn, xp, cpt]
                for st in steps:
                    if defer:
                        bg.append((("q", sc), st))
                    else:
                        st()

            def project_qk(kind, src3, w_sb, dst, sc, sts=None, defer=False):
                """One 512-column group: 8 accumulating matmuls (weights
                stationary); psum copied to the bf16 qT/kT tile."""
                if sts is None:
                    sts = stage_load(src3, sc, E3)
                w = DC // len(sts)
                box = {}

                def mm(dc):
                    if dc == 0:
                        box["ps"] = ps_mm.tile([DK, 512], F32, tag="mm",
                                               name="ps_qk")
                    nc.tensor.matmul(box["ps"][:], w_sb[:, dc, :],
                                     sts[dc // w][:, dc % w, :],
                                     start=(dc == 0), stop=(dc == DC - 1))

                def cp():
                    nc.vector.tensor_copy(dst[:], box["ps"][:])

                steps = [lambda dc=dc: mm(dc) for dc in range(DC)] + [cp]
                for st in steps:
                    if defer:
                        bg.append(((kind, sc), st))
                    else:
                        st()

            def project_v(sc, sts=None, defer=False):
                """V projected directly to natural [s, c] blocks: lhsT is the
                staged data chunk, rhs the weights -> out [128 s, 64 c], which
                is exactly the PV lhsT layout."""
                if sts is None:
                    sts = stage_load(vt3, sc, E3)
                w = DC // len(sts)
                box = {}

                def mm(sb):
                    if sb == 0:
                        box["ps"] = ps_mm.tile([128, 4, DK], F32, tag="mm",
                                               name="ps_v")
                    for dc in range(DC):
                        nc.tensor.matmul(
                            box["ps"][:, sb, :],
                            sts[dc // w][:, dc % w,
                                         sb * 128:(sb + 1) * 128],
                            wv_sb[:, dc, :],
                            start=(dc == 0), stop=(dc == DC - 1))

                def cp():
                    nc.vector.tensor_copy(vgs[sc][:, :, 0:DK], box["ps"][:])
                    nc.vector.memset(vgs[sc][:, :, DK:DK + 1], 1.0)

                steps = [lambda sb=sb: mm(sb) for sb in range(4)] + [cp]
                for st in steps:
                    if defer:
                        bg.append((("v", sc), st))
                    else:
                        st()

            # PE warm-up in the initial DMA shadow
            warm = const.tile([128, 512], BF16, tag="warm")
            nc.vector.memset(warm[:], 0.0)
            for _ in range(4):
                wps = ps_mm.tile([DK, 512], F32, tag="mm", name="wps")
                nc.tensor.matmul(wps[:], warm[:, 0:DK], warm[:],
                                 start=True, stop=True)
            wq_sb = const.tile([128, DC, DK], E3, tag="wq")
            wk_sb = const.tile([128, DC, DK], E3, tag="wk")
            wv_sb = const.tile([128, DC, DK], BF16, tag="wv")
            head_q0 = stage_load(qt3, 0, E3)
            nc.sync.dma_start(wq_sb[:], wq_d.rearrange("(o p) c -> p o c", p=128))
            nc.sync.dma_start(wk_sb[:], wk_d.rearrange("(o p) c -> p o c", p=128))
            nc.sync.dma_start(wv_sb[:], wv_d.rearrange("(o p) c -> p o c", p=128))
            head_k0 = stage_load(kt3, 0, E3)
            head_v0 = stage_load(vt3, 0, E3)
            eye_sb = const.tile([128, 128], BF16, tag="eye")
            nc.sync.dma_start(eye_sb[:], eye_d[:])
            if causal:
                msk_sb = const.tile([JB, CH], BF16, tag="msk")
                nc.sync.dma_start(msk_sb[:], mask_d[:])

            # qT tiles [64, 512] bf16 (2 chunks per tile)
            qts = [res.tile([DK, 512], BF16, tag=f"qt{sc}", name=f"qt{sc}")
                   for sc in range(NSQ)]
            # natural-layout staging for the q transposes, one per group
            qng = [res.tile([128, 4, DK], BF16, tag=f"qn{sc}", name=f"qn{sc}")
                   for sc in range(NSQ)]
            # kT tiles [64, 512] (4 own key blocks per tile)
            kts = [res.tile([DK, 512], BF16, tag=f"kt{sc}", name=f"kt{sc}")
                   for sc in range(NSK)]
            # v natural (+ones col): per 512-group, 4 blocks of [128, 65]
            vgs = [res.tile([128, 4, DK + 1], BF16, tag=f"vg{sc}",
                            name=f"vg{sc}")
                   for sc in range(NSK)]

            def q_rhs(c):
                return qts[c // 2][:, (c % 2) * CH:(c % 2 + 1) * CH]

            def emit_scores(c, m0, nb):
                """Scores + exp (+boundary mask) for one unit; PV deferred so
                the in-order PE isn't parked on a PV that waits for this
                unit's exp -- later scores slot into the exp latency."""
                nb_tot = nkq[c]
                s_ps = ps_s.tile([128, nb, CH], F32, tag="s", name=f"s{nb}")
                for i in range(nb):
                    m = m0 + i
                    nc.tensor.matmul(
                        s_ps[:, i, :],
                        kts[m // 4][:, (m % 4) * JB:(m % 4 + 1) * JB],
                        q_rhs(c), start=True, stop=True)
                if bg:
                    bg.pop(0)[1]()
                    if len(bg) > 10:
                        bg.pop(0)[1]()
                p_sb = pwork.tile([128, nb, CH], BF16, tag="p",
                                  name=f"p{nb}")
                nc.scalar.activation(p_sb[:], s_ps[:],
                                     mybir.ActivationFunctionType.Exp,
                                     scale=SCALE)
                if causal and m0 + nb == nb_tot:
                    # boundary block is always the chunk's last block
                    nc.vector.tensor_mul(p_sb[:, nb - 1, :],
                                         p_sb[:, nb - 1, :], msk_sb[:])
                return (c, m0, nb, p_sb)

            def emit_pv(item, ops, started, left, done):
                c, m0, nb, p_sb = item
                left[c] -= 1
                for i in range(nb):
                    m = m0 + i
                    nc.tensor.matmul(
                        ops[c][:], vgs[m // 4][:, m % 4, :], p_sb[:, i, :],
                        start=(not started[c] and i == 0),
                        stop=(left[c] == 0 and i == nb - 1))
                started[c] = True
                if left[c] == 0:
                    done.append(c)

            def epilogue(c, o_ps):
                # POOL DGE so result stores don't head-of-line block the SP
                # sequencer issuing input stage loads
                o_sb = pwork.tile([DK + 1, CH], F32, tag="osb", name="o_sb")
                nc.vector.tensor_copy(o_sb[:], o_ps[:])
                eng = nc.sync if c >= NCH - 2 else nc.gpsimd
                eng.dma_start(out_d[c], o_sb[:])

            def chunk_pair(c0, c1):
                """Interleave the score/exp/PV units of two chunks so one
                chunk's PE work hides the other's ACT-exp latency (each chunk
                accumulates in its own PSUM bank)."""
                cs = [c for c in (c0, c1) if c is not None]
                units = {c: [(m0, min(4, nkq[c] - m0))
                             for m0 in range(0, nkq[c], 4)] for c in cs}
                # boundary (masked) unit first, so each chunk's closing
                # chain has no mask op and the masked exp starts earliest
                for c in cs:
                    units[c] = units[c][-1:] + units[c][:-1]
                ops = {c: ps_o.tile([DK + 1, CH], F32, tag="o",
                                    name=f"o{c % 2}") for c in cs}
                started = {c: False for c in cs}
                left = {c: len(units[c]) for c in cs}
                pend = []
                done = []
                nu = max(len(units[c]) for c in cs)
                for u in range(nu):
                    for c in cs:
                        if u < len(units[c]):
                            m0, nb = units[c][u]
                            pend.append(emit_scores(c, m0, nb))
                            while len(pend) > 6:
                                emit_pv(pend.pop(0), ops, started, left, done)
                while pend:
                    emit_pv(pend.pop(0), ops, started, left, done)
                for c in done:
                    epilogue(c, ops[c])

            # projection prefetch schedule: kT pair tile m+1 (k group
            # (m+1)//4) is needed by chunk m, so k group g feeds chunks
            # >= 4g-1; v group g feeds >= 4g; q group g feeds >= 2g
            pre = {c: [] for c in range(NCH)}
            for g in range(1, NSK):
                pre[max(0, 4 * g - 3)] += [("k", g), ("v", g)]
            for g in range(1, NSQ):  # q groups 1..7 needed at chunk 2g
                pre[max(0, 2 * g - 3)] += [("q", g)]

            project_qk("q", qt3, wq_sb, qts[0], 0, sts=head_q0)
            project_qk("k", kt3, wk_sb, kts[0], 0, sts=head_k0)
            project_v(0, sts=head_v0)
            for c0 in range(0, NCH, 2):
                c1 = c0 + 1
                # groups the current pair depends on must be fully emitted
                needed = {("q", g) for g in range(c1 // 2 + 1)}
                needed |= {("k", g) for g in range(c1 // 4 + 1)}
                needed |= {("v", g) for g in range(c1 // 4 + 1)}
                while any(k in needed for k, _ in bg):
                    bg.pop(0)[1]()
                # stage + enqueue projections for upcoming chunks; their
                # matmuls are woven between this pair's attention units
                for c in (c0, c1):
                    for kind, g in pre[c]:
                        if kind == "q":
                            project_q(g, defer=True)
                        elif kind == "k":
                            project_qk("k", kt3, wk_sb, kts[g], g, defer=True)
                        else:
                            project_v(g, defer=True)
                chunk_pair(c0, c1)
            while bg:
                bg.pop(0)[1]()

    nc.compile()
    return nc


def _get_prog(variant):
    if variant not in _prog_cache:
        _prog_cache[variant] = _build(variant)
    return _prog_cache[variant]


def kernel(queries, keys, values, Wq, Wk, Wv, mask):
    import ml_dtypes  # noqa: F401  registers numpy bfloat16/fp8

    bf16 = np.dtype(mybir.dt.np(BF16))
    e3m4 = np.dtype(mybir.dt.np(E3))
    queries = np.asarray(queries, dtype=np.float32)
    keys = np.asarray(keys, dtype=np.float32)
    values = np.asarray(values, dtype=np.float32)
    mask_np = np.asarray(mask)

    causal = bool(np.array_equal(
        mask_np != 0, np.tril(np.ones((S, S), dtype=bool))))
    full = bool((mask_np != 0).all()) if not causal else False
    if not (causal or full):
        raise NotImplementedError("general mask not supported")
    variant = "causal" if causal else "full"

    qt = np.ascontiguousarray(queries.transpose(0, 2, 1)).astype(e3m4)
    kt = np.ascontiguousarray(keys.transpose(0, 2, 1)).astype(e3m4)
    vt = np.ascontiguousarray(values.transpose(0, 2, 1)).astype(e3m4)
    wq = np.ascontiguousarray(
        np.asarray(Wq, dtype=np.float32).T * WSCALE).astype(e3m4)
    wk = np.ascontiguousarray(
        np.asarray(Wk, dtype=np.float32).T * WSCALE).astype(e3m4)
    wv = np.ascontiguousarray(np.asarray(Wv, dtype=np.float32).T).astype(bf16)

    in_maps = []
    for core in range(NCORES):
        b, h = divmod(core, 2)
        ksel = np.ascontiguousarray(
            kt[b].reshape(D, S // JB, JB)[:, h::2, :].reshape(D, KB))
        vsel = np.ascontiguousarray(
            vt[b].reshape(D, S // JB, JB)[:, h::2, :].reshape(D, KB))
        m = {"qt": qt[b], "kt": ksel, "vt": vsel,
             "wq": wq, "wk": wk, "wv": wv,
             "eye": np.eye(128, dtype=np.float32).astype(bf16)}
        if variant == "causal":
            i = np.arange(CH)[None, :]
            j = np.arange(JB)[:, None]
            m["maskb"] = ((i - j - JB * h) >= 0).astype(np.float32).astype(bf16)
        in_maps.append(m)

    nc = _get_prog(variant)
    res = run_bass_kernel_spmd(nc, in_maps, list(range(NCORES)))

    out = np.empty((B, S, DK), dtype=np.float32)
    ov = out.reshape(B, NCH, CH, DK)
    for b in range(B):
        r0 = res.results[2 * b]["out"]  # [NCH, DK+1, CH]
        r1 = res.results[2 * b + 1]["out"]
        tot = r0.astype(np.float64) + r1.astype(np.float64)
        ov[b] = (tot[:, :DK, :] / tot[:, DK:DK + 1, :]).transpose(0, 2, 1)
    return out


if __name__ == "__main__":
    rng = np.random.default_rng(0)
    q = rng.standard_normal((B, S, D), dtype=np.float32)
    k = rng.standard_normal((B, S, D), dtype=np.float32)
    v = rng.standard_normal((B, S, D), dtype=np.float32)
    sc = 1.0 / np.sqrt(D)
    wq = rng.uniform(-sc, sc, (DK, D)).astype(np.float32)
    wk = rng.uniform(-sc, sc, (DK, D)).astype(np.float32)
    wv = rng.uniform(-sc, sc, (DK, D)).astype(np.float32)
    msk = np.tril(np.ones((S, S), dtype=np.int32))
    out = kernel(queries=q, keys=k, values=v, Wq=wq, Wk=wk, Wv=wv, mask=msk)
    print("out", out.shape, out.dtype, float(np.abs(out).mean()))


# revision 60
# speedup vs baseline: 1.0679x; 1.0092x over previous
"""Causal single-head attention (B=4, S=4096, D=1024, d_key=64) on 8 trn2 cores.

Sharding: 8 cores = 4 batches x 2 KEY-halves. Core (b, h) holds ALL 4096 query
rows of batch b but only the key/value 128-row blocks {j : j % 2 == h} (2048
keys, interleaved for causal balance). Each core computes the partial softmax
accumulator (unnormalized numerator + denominator row) of every query row over
its own key half; the HOST adds the two halves of each pair and normalizes.
No cross-core communication, and K/V raw loads + projections are not
replicated (the baseline replicated both).

DMA diet: queries, keys AND values stream in as fp8 e3m4 (4 mantissa bits);
the projection weights Wq/Wk are pre-scaled by 64 on the host so their range
suits e3m4, and the 1/64^2 is folded into the softmax exp scale. Wv stays
bf16 (mixed e3m4-lhsT x bf16-rhs matmuls work). Measured end-to-end l2
rel-err 1.55e-2 vs fp64 (gate 2e-2): ~1.0e-2 from the q/k score path,
~1.1e-2 from the V path, in quadrature.

Device kernel (identical SPMD program; per-core differences are input data):
  1. Project qT [64, 4096] and kT [64, 2048] (weights stationary, e3m4 data,
     fp32 PSUM, stored bf16) and v-natural [128, 65]-blocks (data stationary
     -> natural PV layout; col 64 is a ones column for the denominator).
     Projections are split into per-matmul thunks and woven one-or-two at a
     time between attention units (the `bg` queue) so the in-order PE stays
     fed while stage DMAs pace the input.
  2. CHUNK-major attention, two chunks interleaved per pair (each with its
     own PSUM accumulator bank): for q chunk c (256 rows), own-key blocks
     m=0..c (the packed block m maps to global block 2m+h, so the count and
     boundary structure are core-independent): score matmuls in units of up
     to 4 blocks -> one ACT exp per unit -> boundary mask (a single constant
     [128,256] tile, only block m==c needs it) -> PV matmuls accumulate the
     whole chunk in one PSUM tile [65, 256]. PV emission is DEFERRED behind
     a 6-unit window so the in-order PE never parks on a PV waiting for its
     exp -- later units' scores fill the ACT latency instead.
  3. Per chunk the accumulator is copied to SBUF and DMAd out via the POOL
     DGE (last two chunks via HWDGE); host sums the two half outputs of each
     batch pair, normalizes by the denominator row, and transposes.
"""

import numpy as np

import concourse.mybir as mybir
import concourse.tile as tile
from concourse import bacc
from concourse.bass_utils import run_bass_kernel_spmd

B, S, D, DK = 4, 4096, 1024, 64
NCORES = 8
CH = 256  # query rows per chunk
NCH = 16  # chunks per core (all rows)
KB = 2048  # own keys per core
JB = 128  # key block
NKB = KB // JB  # 16 own key blocks
DC = D // 128  # 8 contraction chunks
F32 = mybir.dt.float32
BF16 = mybir.dt.bfloat16
E3 = mybir.dt.float8e3
WSCALE = 64.0  # host pre-scales Wq/Wk by this; folded into exp scale
SCALE = 0.125 / (WSCALE * WSCALE)

_prog_cache = {}


def _build(variant):
    causal = variant == "causal"
    nkq = [c + 1 if causal else NKB for c in range(NCH)]  # own blocks/chunk

    nc = bacc.Bacc("TRN2", target_bir_lowering=False, debug=False,
                   num_devices=NCORES)

    qt_d = nc.declare_dram_parameter("qt", [D, S], E3, isOutput=False)
    kt_d = nc.declare_dram_parameter("kt", [D, KB], E3, isOutput=False)
    vt_d = nc.declare_dram_parameter("vt", [D, KB], E3, isOutput=False)
    wq_d = nc.declare_dram_parameter("wq", [D, DK], E3, isOutput=False)
    wk_d = nc.declare_dram_parameter("wk", [D, DK], E3, isOutput=False)
    wv_d = nc.declare_dram_parameter("wv", [D, DK], BF16, isOutput=False)
    if causal:
        mask_d = nc.declare_dram_parameter("maskb", [JB, CH], BF16,
                                           isOutput=False)
    # raw transposed partial accumulators (+denominator row); host combines
    out_d = nc.declare_dram_parameter("out", [NCH, DK + 1, CH], F32,
                                      isOutput=True)

    NSQ = S // 512  # 8 column groups of 512 for q
    NSK = KB // 512  # 4 groups for k/v

    qt3 = qt_d.rearrange("(o p) s -> p o s", p=128)
    kt3 = kt_d.rearrange("(o p) s -> p o s", p=128)
    vt3 = vt_d.rearrange("(o p) s -> p o s", p=128)

    with tile.TileContext(nc) as tc:
        with (
            tc.tile_pool(name="const", bufs=1) as const,
            tc.tile_pool(name="res", bufs=1) as res,
            tc.tile_pool(name="stage", bufs=12) as stage,
            tc.tile_pool(name="pwork", bufs=10) as pwork,
            tc.tile_pool(name="ps_mm", bufs=2, space="PSUM") as ps_mm,
            tc.tile_pool(name="ps_s", bufs=2, space="PSUM") as ps_s,
            tc.tile_pool(name="ps_o", bufs=2, space="PSUM") as ps_o,
        ):
            def stage_load(src3, sc, dt, splits=2):
                """Split-group DMAs so the first matmuls start early."""
                w = DC // splits
                sts = []
                for hh in range(splits):
                    st = stage.tile([128, w, 512], dt, tag="stage",
                                    name=f"st{hh}")
                    nc.sync.dma_start(
                        st[:],
                        src3[:, w * hh:w * (hh + 1), sc * 512:(sc + 1) * 512])
                    sts.append(st)
                return sts

            bg = []  # background projection thunks, woven between attn units

            def project_qk(kind, src3, w_sb, dst, sc, sts=None, defer=False):
                """One 512-column group: 8 accumulating matmuls (weights
                stationary); psum copied to the bf16 qT/kT tile."""
                if sts is None:
                    sts = stage_load(src3, sc, E3)
                w = DC // len(sts)
                box = {}

                def mm(dc):
                    if dc == 0:
                        box["ps"] = ps_mm.tile([DK, 512], F32, tag="mm",
                                               name="ps_qk")
                    nc.tensor.matmul(box["ps"][:], w_sb[:, dc, :],
                                     sts[dc // w][:, dc % w, :],
                                     start=(dc == 0), stop=(dc == DC - 1))

                def cp():
                    nc.vector.tensor_copy(dst[:], box["ps"][:])

                steps = [lambda dc=dc: mm(dc) for dc in range(DC)] + [cp]
                for st in steps:
                    if defer:
                        bg.append(((kind, sc), st))
                    else:
                        st()

            def project_v(sc, sts=None, defer=False):
                """V projected directly to natural [s, c] blocks: lhsT is the
                staged data chunk, rhs the weights -> out [128 s, 64 c], which
                is exactly the PV lhsT layout."""
                if sts is None:
                    sts = stage_load(vt3, sc, E3)
                w = DC // len(sts)
                box = {}

                def mm(sb):
                    if sb == 0:
                        box["ps"] = ps_mm.tile([128, 4, DK], F32, tag="mm",
                                               name="ps_v")
                    for dc in range(DC):
                        nc.tensor.matmul(
                            box["ps"][:, sb, :],
                            sts[dc // w][:, dc % w,
                                         sb * 128:(sb + 1) * 128],
                            wv_sb[:, dc, :],
                            start=(dc == 0), stop=(dc == DC - 1))

                def cp():
                    nc.vector.tensor_copy(vgs[sc][:, :, 0:DK], box["ps"][:])
                    nc.vector.memset(vgs[sc][:, :, DK:DK + 1], 1.0)

                steps = [lambda sb=sb: mm(sb) for sb in range(4)] + [cp]
                for st in steps:
                    if defer:
                        bg.append((("v", sc), st))
                    else:
                        st()

            # PE warm-up in the initial DMA shadow
            warm = const.tile([128, 512], BF16, tag="warm")
            nc.vector.memset(warm[:], 0.0)
            for _ in range(4):
                wps = ps_mm.tile([DK, 512], F32, tag="mm", name="wps")
                nc.tensor.matmul(wps[:], warm[:, 0:DK], warm[:],
                                 start=True, stop=True)
            wq_sb = const.tile([128, DC, DK], E3, tag="wq")
            wk_sb = const.tile([128, DC, DK], E3, tag="wk")
            wv_sb = const.tile([128, DC, DK], BF16, tag="wv")
            head_q0 = stage_load(qt3, 0, E3)
            nc.sync.dma_start(wq_sb[:], wq_d.rearrange("(o p) c -> p o c", p=128))
            nc.sync.dma_start(wk_sb[:], wk_d.rearrange("(o p) c -> p o c", p=128))
            nc.sync.dma_start(wv_sb[:], wv_d.rearrange("(o p) c -> p o c", p=128))
            head_k0 = stage_load(kt3, 0, E3)
            head_v0 = stage_load(vt3, 0, E3)
            if causal:
                msk_sb = const.tile([JB, CH], BF16, tag="msk")
                nc.sync.dma_start(msk_sb[:], mask_d[:])

            # qT tiles [64, 512] bf16 (2 chunks per tile)
            qts = [res.tile([DK, 512], BF16, tag=f"qt{sc}", name=f"qt{sc}")
                   for sc in range(NSQ)]
            # kT tiles [64, 512] (4 own key blocks per tile)
            kts = [res.tile([DK, 512], BF16, tag=f"kt{sc}", name=f"kt{sc}")
                   for sc in range(NSK)]
            # v natural (+ones col): per 512-group, 4 blocks of [128, 65]
            vgs = [res.tile([128, 4, DK + 1], BF16, tag=f"vg{sc}",
                            name=f"vg{sc}")
                   for sc in range(NSK)]

            def q_rhs(c):
                return qts[c // 2][:, (c % 2) * CH:(c % 2 + 1) * CH]

            def emit_scores(c, m0, nb):
                """Scores + exp (+boundary mask) for one unit; PV deferred so
                the in-order PE isn't parked on a PV that waits for this
                unit's exp -- later scores slot into the exp latency."""
                nb_tot = nkq[c]
                s_ps = ps_s.tile([128, nb, CH], F32, tag="s", name=f"s{nb}")
                for i in range(nb):
                    m = m0 + i
                    nc.tensor.matmul(
                        s_ps[:, i, :],
                        kts[m // 4][:, (m % 4) * JB:(m % 4 + 1) * JB],
                        q_rhs(c), start=True, stop=True)
                if bg:
                    bg.pop(0)[1]()
                    if len(bg) > 10:
                        bg.pop(0)[1]()
                p_sb = pwork.tile([128, nb, CH], BF16, tag="p",
                                  name=f"p{nb}")
                nc.scalar.activation(p_sb[:], s_ps[:],
                                     mybir.ActivationFunctionType.Exp,
                                     scale=SCALE)
                if causal and m0 + nb == nb_tot:
                    # boundary block is always the chunk's last block
                    nc.vector.tensor_mul(p_sb[:, nb - 1, :],
                                         p_sb[:, nb - 1, :], msk_sb[:])
                return (c, m0, nb, p_sb)

            def emit_pv(item, ops, started, left, done):
                c, m0, nb, p_sb = item
                left[c] -= 1
                for i in range(nb):
                    m = m0 + i
                    nc.tensor.matmul(
                        ops[c][:], vgs[m // 4][:, m % 4, :], p_sb[:, i, :],
                        start=(not started[c] and i == 0),
                        stop=(left[c] == 0 and i == nb - 1))
                started[c] = True
                if left[c] == 0:
                    done.append(c)

            def epilogue(c, o_ps):
                # POOL DGE so result stores don't head-of-line block the SP
                # sequencer issuing input stage loads
                o_sb = pwork.tile([DK + 1, CH], F32, tag="osb", name="o_sb")
                nc.vector.tensor_copy(o_sb[:], o_ps[:])
                eng = nc.sync if c >= NCH - 2 else nc.gpsimd
                eng.dma_start(out_d[c], o_sb[:])

            def chunk_pair(c0, c1):
                """Interleave the score/exp/PV units of two chunks so one
                chunk's PE work hides the other's ACT-exp latency (each chunk
                accumulates in its own PSUM bank)."""
                cs = [c for c in (c0, c1) if c is not None]
                units = {c: [(m0, min(4, nkq[c] - m0))
                             for m0 in range(0, nkq[c], 4)] for c in cs}
                # boundary (masked) unit first, so each chunk's closing
                # chain has no mask op and the masked exp starts earliest
                for c in cs:
                    units[c] = units[c][-1:] + units[c][:-1]
                ops = {c: ps_o.tile([DK + 1, CH], F32, tag="o",
                                    name=f"o{c % 2}") for c in cs}
                started = {c: False for c in cs}
                left = {c: len(units[c]) for c in cs}
                pend = []
                done = []
                nu = max(len(units[c]) for c in cs)
                for u in range(nu):
                    for c in cs:
                        if u < len(units[c]):
                            m0, nb = units[c][u]
                            pend.append(emit_scores(c, m0, nb))
                            while len(pend) > 6:
                                emit_pv(pend.pop(0), ops, started, left, done)
                while pend:
                    emit_pv(pend.pop(0), ops, started, left, done)
                for c in done:
                    epilogue(c, ops[c])

            # projection prefetch schedule: kT pair tile m+1 (k group
            # (m+1)//4) is needed by chunk m, so k group g feeds chunks
            # >= 4g-1; v group g feeds >= 4g; q group g feeds >= 2g
            pre = {c: [] for c in range(NCH)}
            for g in range(1, NSK):
                pre[max(0, 4 * g - 3)] += [("k", g), ("v", g)]
            for g in range(1, NSQ):  # q groups 1..7 needed at chunk 2g
                pre[max(0, 2 * g - 2)] += [("q", g)]

            project_qk("q", qt3, wq_sb, qts[0], 0, sts=head_q0)
            project_qk("k", kt3, wk_sb, kts[0], 0, sts=head_k0)
            project_v(0, sts=head_v0)
            for c0 in range(0, NCH, 2):
                c1 = c0 + 1
                # groups the current pair depends on must be fully emitted
                needed = {("q", g) for g in range(c1 // 2 + 1)}
                needed |= {("k", g) for g in range(c1 // 4 + 1)}
                needed |= {("v", g) for g in range(c1 // 4 + 1)}
                while any(k in needed for k, _ in bg):
                    bg.pop(0)[1]()
                # stage + enqueue projections for upcoming chunks; their
                # matmuls are woven between this pair's attention units
                for c in (c0, c1):
                    for kind, g in pre[c]:
                        if kind == "q":
                            project_qk("q", qt3, wq_sb, qts[g], g, defer=True)
                        elif kind == "k":
                            project_qk("k", kt3, wk_sb, kts[g], g, defer=True)
                        else:
                            project_v(g, defer=True)
                chunk_pair(c0, c1)
            while bg:
                bg.pop(0)[1]()

    nc.compile()
    return nc


def _get_prog(variant):
    if variant not in _prog_cache:
        _prog_cache[variant] = _build(variant)
    return _prog_cache[variant]


def kernel(queries, keys, values, Wq, Wk, Wv, mask):
    import ml_dtypes  # noqa: F401  registers numpy bfloat16/fp8

    bf16 = np.dtype(mybir.dt.np(BF16))
    e3m4 = np.dtype(mybir.dt.np(E3))
    queries = np.asarray(queries, dtype=np.float32)
    keys = np.asarray(keys, dtype=np.float32)
    values = np.asarray(values, dtype=np.float32)
    mask_np = np.asarray(mask)

    causal = bool(np.array_equal(
        mask_np != 0, np.tril(np.ones((S, S), dtype=bool))))
    full = bool((mask_np != 0).all()) if not causal else False
    if not (causal or full):
        raise NotImplementedError("general mask not supported")
    variant = "causal" if causal else "full"

    qt = np.ascontiguousarray(queries.transpose(0, 2, 1)).astype(e3m4)
    kt = np.ascontiguousarray(keys.transpose(0, 2, 1)).astype(e3m4)
    vt = np.ascontiguousarray(values.transpose(0, 2, 1)).astype(e3m4)
    wq = np.ascontiguousarray(
        np.asarray(Wq, dtype=np.float32).T * WSCALE).astype(e3m4)
    wk = np.ascontiguousarray(
        np.asarray(Wk, dtype=np.float32).T * WSCALE).astype(e3m4)
    wv = np.ascontiguousarray(np.asarray(Wv, dtype=np.float32).T).astype(bf16)

    in_maps = []
    for core in range(NCORES):
        b, h = divmod(core, 2)
        ksel = np.ascontiguousarray(
            kt[b].reshape(D, S // JB, JB)[:, h::2, :].reshape(D, KB))
        vsel = np.ascontiguousarray(
            vt[b].reshape(D, S // JB, JB)[:, h::2, :].reshape(D, KB))
        m = {"qt": qt[b], "kt": ksel, "vt": vsel,
             "wq": wq, "wk": wk, "wv": wv}
        if variant == "causal":
            i = np.arange(CH)[None, :]
            j = np.arange(JB)[:, None]
            m["maskb"] = ((i - j - JB * h) >= 0).astype(np.float32).astype(bf16)
        in_maps.append(m)

    nc = _get_prog(variant)
    res = run_bass_kernel_spmd(nc, in_maps, list(range(NCORES)))

    out = np.empty((B, S, DK), dtype=np.float32)
    ov = out.reshape(B, NCH, CH, DK)
    for b in range(B):
        r0 = res.results[2 * b]["out"]  # [NCH, DK+1, CH]
        r1 = res.results[2 * b + 1]["out"]
        tot = r0.astype(np.float64) + r1.astype(np.float64)
        ov[b] = (tot[:, :DK, :] / tot[:, DK:DK + 1, :]).transpose(0, 2, 1)
    return out


if __name__ == "__main__":
    rng = np.random.default_rng(0)
    q = rng.standard_normal((B, S, D), dtype=np.float32)
    k = rng.standard_normal((B, S, D), dtype=np.float32)
    v = rng.standard_normal((B, S, D), dtype=np.float32)
    sc = 1.0 / np.sqrt(D)
    wq = rng.uniform(-sc, sc, (DK, D)).astype(np.float32)
    wk = rng.uniform(-sc, sc, (DK, D)).astype(np.float32)
    wv = rng.uniform(-sc, sc, (DK, D)).astype(np.float32)
    msk = np.tril(np.ones((S, S), dtype=np.int32))
    out = kernel(queries=q, keys=k, values=v, Wq=wq, Wk=wk, Wv=wv, mask=msk)
    print("out", out.shape, out.dtype, float(np.abs(out).mean()))
